# revision 3
# baseline (speedup 1.0000x reference)
"""LoRA multi-head attention on 8 Trainium2 NeuronCores.

Sharding: data-parallel over batch (B=2) x tensor-parallel over heads
(16 heads -> 4 per core).  Core c handles batch b=c//4 and head group
g=c%4 (columns C=[256*g, 256*g+256) of the projection output).

Host prep (per weight): W_eff = W + 2.0 * B @ A  (exact LoRA fold),
and transposed activations x.T so the contraction dim lands on SBUF
partitions.  Device computes, per core:
  Q^T, K^T  [256, 2048]   (C-slice of heads, d_k on partitions)
  V         [2048, 256]   (natural layout, per-head 65-wide with a
                           ones column folded in for softmax row sums)
  scoresT   [t, q] tiles  -> exp on ACT (scale=1/8) -> PV matmuls
  ctx^T     [256, 2048]   normalized by the folded row sums
  outT      [1024, 2048]  partial output projection (summed on host
                           across the 4 cores of each batch group)
All matmuls run as float32r (fp32 storage, full-rate PE).
PSUM budget (8 banks): proj/out-proj rounds 2, scoresT 2x[128,1024]=4,
ctx pair 2.
"""

import sys

sys.path.insert(0, "/opt/trn_rl_repo")

from contextlib import ExitStack

import numpy as np

import concourse.bass as bass
import concourse.tile as tile
from concourse import bacc, mybir
from concourse.bass_utils import run_bass_kernel_spmd

F32 = mybir.dt.float32
F32R = mybir.dt.float32r

B = 2
S = 2048
D = 1024
H = 16
DK = 64
SCALING = 2.0
N_CORES = 8
CPG = 4  # cores per batch group
CSLICE = D // CPG  # 256 columns of head-dim per core
Exp = mybir.ActivationFunctionType.Exp
MULT = mybir.AluOpType.mult

_CACHE = {}


def _build():
    nc = bacc.Bacc("TRN2", target_bir_lowering=False, debug=False)

    xqT = nc.declare_dram_parameter("xqT", [D, S], F32R, isOutput=False)
    xkT = nc.declare_dram_parameter("xkT", [D, S], F32R, isOutput=False)
    xvT = nc.declare_dram_parameter("xvT", [D, S], F32R, isOutput=False)
    wq = nc.declare_dram_parameter("wq", [D, CSLICE], F32R, isOutput=False)
    wk = nc.declare_dram_parameter("wk", [D, CSLICE], F32R, isOutput=False)
    wv = nc.declare_dram_parameter("wv", [D, CSLICE], F32R, isOutput=False)
    wo = nc.declare_dram_parameter("wo", [CSLICE, D], F32R, isOutput=False)
    outT = nc.declare_dram_parameter("outT", [D, S], F32, isOutput=True)

    with tile.TileContext(nc) as tc, ExitStack() as ctx:
        const = ctx.enter_context(tc.tile_pool(name="const", bufs=1))
        xp = ctx.enter_context(tc.tile_pool(name="xp", bufs=10))
        expp = ctx.enter_context(tc.tile_pool(name="expp", bufs=3))
        smallp = ctx.enter_context(tc.tile_pool(name="smallp", bufs=2))
        psum = ctx.enter_context(tc.tile_pool(name="psum", bufs=2, space="PSUM"))

        # ---- resident tensors -------------------------------------------
        wq_sb = const.tile([128, 8, CSLICE], F32R)
        wk_sb = const.tile([128, 8, CSLICE], F32R)
        wv_sb = const.tile([128, 8, CSLICE], F32R)
        wo_sb = const.tile([128, 2, D], F32R)
        nc.sync.dma_start(wq_sb[:], wq.rearrange("(i p) c -> p i c", p=128))
        nc.sync.dma_start(wk_sb[:], wk.rearrange("(i p) c -> p i c", p=128))
        nc.sync.dma_start(wv_sb[:], wv.rearrange("(i p) c -> p i c", p=128))
        nc.sync.dma_start(wo_sb[:], wo.rearrange("(c p) o -> p c o", p=128))

        qT_sb = const.tile([128, 2, S], F32R)
        kT_sb = const.tile([128, 2, S], F32R)
        v_sb = const.tile([128, 16, 4, DK + 1], F32R)
        ctxT_sb = const.tile([128, 2, S], F32R)

        ones_f = const.tile([128, 64], F32)
        nc.vector.memset(ones_f[:], 1.0)
        nc.vector.tensor_copy(
            v_sb[:, :, :, DK : DK + 1],
            ones_f[:].rearrange("p (a b c) -> p a b c", a=16, b=4, c=1),
        )

        # ---- Q / K projection passes ------------------------------------
        for xsrc, wsb, dst in ((xqT, wq_sb, qT_sb), (xkT, wk_sb, kT_sb)):
            for sb2 in range(2):  # s-halves of 1024
                xts = []
                for i in range(8):
                    xt = xp.tile([128, 1024], F32R, tag="xt")
                    nc.sync.dma_start(
                        xt[:],
                        xsrc[128 * i : 128 * (i + 1), 1024 * sb2 : 1024 * (sb2 + 1)],
                    )
                    xts.append(xt)
                for cc in range(2):
                    for st2 in range(2):
                        ps = psum.tile([128, 512], F32, tag="pj", bufs=2)
                        for i in range(8):
                            nc.tensor.matmul(
                                ps[:],
                                wsb[:, i, 128 * cc : 128 * (cc + 1)],
                                xts[i][:, 512 * st2 : 512 * (st2 + 1)],
                                start=(i == 0),
                                stop=(i == 7),
                            )
                        off = 1024 * sb2 + 512 * st2
                        nc.vector.tensor_copy(dst[:, cc, off : off + 512], ps[:])

        # ---- V projection pass (natural layout, per-head 65-wide) -------
        for tth in range(2):  # 1024-row t-halves
            xts = []
            for i in range(8):
                xt = xp.tile([128, 1024], F32R, tag="xt")
                nc.sync.dma_start(
                    xt[:],
                    xvT[128 * i : 128 * (i + 1), 1024 * tth : 1024 * (tth + 1)],
                )
                xts.append(xt)
            for t8 in range(8):  # 128-row t tiles within the half
                ps = psum.tile([128, 256], F32, tag="pj", bufs=2)
                for i in range(8):
                    nc.tensor.matmul(
                        ps[:],
                        xts[i][:, 128 * t8 : 128 * (t8 + 1)],
                        wv_sb[:, i, :],
                        start=(i == 0),
                        stop=(i == 7),
                    )
                nc.vector.tensor_copy(
                    v_sb[:, 8 * tth + t8, :, 0:DK],
                    ps[:].rearrange("p (h d) -> p h d", h=4),
                )

        # ---- attention + output projection ------------------------------
        for qt in range(4):  # 512-wide q tiles
            qs = slice(512 * qt, 512 * (qt + 1))
            for p in range(2):  # head pairs (2p, 2p+1)
                ctx0 = psum.tile([DK + 1, 512], F32, tag="ctx", bufs=2)
                ctx1 = psum.tile([DK + 1, 512], F32, tag="ctx", bufs=2)
                for t in range(16):
                    sc = psum.tile([128, 1024], F32, tag="sc", bufs=2)
                    ts_ = slice(128 * t, 128 * (t + 1))
                    nc.tensor.matmul(
                        sc[:, 0:512],
                        kT_sb[0:64, p, ts_],
                        qT_sb[0:64, p, qs],
                        start=True,
                        stop=True,
                        tile_position=(0, 0),
                    )
                    nc.tensor.matmul(
                        sc[:, 512:1024],
                        kT_sb[64:128, p, ts_],
                        qT_sb[64:128, p, qs],
                        start=True,
                        stop=True,
                        tile_position=(64, 0),
                    )
                    et = expp.tile([128, 1024], F32R)
                    nc.scalar.activation(et[:], sc[:], Exp, scale=1.0 / 8.0)
                    nc.tensor.matmul(
                        ctx0[:],
                        v_sb[:, t, 2 * p, :],
                        et[:, 0:512],
                        start=(t == 0),
                        stop=(t == 15),
                    )
                    nc.tensor.matmul(
                        ctx1[:],
                        v_sb[:, t, 2 * p + 1, :],
                        et[:, 512:1024],
                        start=(t == 0),
                        stop=(t == 15),
                    )
                for h01, cx in ((0, ctx0), (1, ctx1)):
                    rc = smallp.tile([1, 512], F32, tag="rc")
                    nc.vector.reciprocal(rc[:], cx[DK : DK + 1, :])
                    bc = smallp.tile([64, 512], F32, tag="bc")
                    nc.gpsimd.partition_broadcast(bc[:], rc[:])
                    ct = smallp.tile([64, 512], F32R, tag="ct")
                    nc.vector.tensor_tensor(ct[:], cx[0:DK, :], bc[:], MULT)
                    nc.sync.dma_start(ctxT_sb[64 * h01 : 64 * h01 + 64, p, qs], ct[:])
            for o in range(8):
                ops = psum.tile([128, 512], F32, tag="pj", bufs=2)
                nc.tensor.matmul(
                    ops[:],
                    wo_sb[:, 0, 128 * o : 128 * (o + 1)],
                    ctxT_sb[:, 0, qs],
                    start=True,
                    stop=False,
                )
                nc.tensor.matmul(
                    ops[:],
                    wo_sb[:, 1, 128 * o : 128 * (o + 1)],
                    ctxT_sb[:, 1, qs],
                    start=False,
                    stop=True,
                )
                ob = smallp.tile([128, 512], F32, tag="ob", bufs=3)
                nc.vector.tensor_copy(ob[:], ops[:])
                nc.sync.dma_start(outT[128 * o : 128 * (o + 1), qs], ob[:])

    nc.finalize()
    return nc


def _get_nc():
    if "nc" not in _CACHE:
        _CACHE["nc"] = _build()
    return _CACHE["nc"]


def _numpy_reference(query, key, value, mask, Wq, Aq, Bq, Wk, Ak, Bk, Wv, Av, Bv, Wo, Ao, Bo):
    """Exact fallback for a non-all-ones mask (never hit for the spec'd inputs)."""

    def lora(x, W, A, Bm):
        return x @ W.T + ((x @ A.T) @ Bm.T) * SCALING

    q = lora(query, Wq, Aq, Bq).reshape(B, S, H, DK).transpose(0, 2, 1, 3)
    k = lora(key, Wk, Ak, Bk).reshape(B, S, H, DK).transpose(0, 2, 1, 3)
    v = lora(value, Wv, Av, Bv).reshape(B, S, H, DK).transpose(0, 2, 1, 3)
    sc = np.einsum("bhqd,bhkd->bhqk", q, k) / np.sqrt(DK).astype(np.float32)
    sc = np.where(mask == 0, np.float32(-1e9), sc)
    sc = sc - sc.max(axis=-1, keepdims=True)
    e = np.exp(sc)
    attn = e / e.sum(axis=-1, keepdims=True)
    cx = np.einsum("bhqk,bhkd->bhqd", attn, v)
    cx = cx.transpose(0, 2, 1, 3).reshape(B, S, D)
    return lora(cx, Wo, Ao, Bo).astype(np.float32)


def _prepare_in_maps(query, key, value, Wq, Aq, Bq, Wk, Ak, Bk, Wv, Av, Bv, Wo, Ao, Bo):
    f32 = np.float32
    weff = {}
    for n, (W, A, Bm) in {
        "q": (Wq, Aq, Bq),
        "k": (Wk, Ak, Bk),
        "v": (Wv, Av, Bv),
        "o": (Wo, Ao, Bo),
    }.items():
        weff[n] = (
            np.asarray(W, f32) + SCALING * np.asarray(Bm, f32) @ np.asarray(A, f32)
        ).astype(f32)

    xT = {
        "q": [np.ascontiguousarray(np.asarray(query[b], f32).T) for b in range(B)],
        "k": [np.ascontiguousarray(np.asarray(key[b], f32).T) for b in range(B)],
        "v": [np.ascontiguousarray(np.asarray(value[b], f32).T) for b in range(B)],
    }
    in_maps = []
    for c in range(N_CORES):
        b, g = divmod(c, CPG)
        cs = slice(CSLICE * g, CSLICE * (g + 1))
        in_maps.append(
            {
                "xqT": xT["q"][b],
                "xkT": xT["k"][b],
                "xvT": xT["v"][b],
                "wq": np.ascontiguousarray(weff["q"][cs, :].T),
                "wk": np.ascontiguousarray(weff["k"][cs, :].T),
                "wv": np.ascontiguousarray(weff["v"][cs, :].T),
                "wo": np.ascontiguousarray(weff["o"][:, cs].T),
            }
        )
    return in_maps


def run(inputs, trace=False, **spmd_kwargs):
    """Shard, run on 8 cores, gather.  Returns (output, BassKernelResults)."""
    mask = np.asarray(inputs["mask"])
    if not np.all(mask != 0):
        out = _numpy_reference(
            np.asarray(inputs["query"], np.float32),
            np.asarray(inputs["key"], np.float32),
            np.asarray(inputs["value"], np.float32),
            mask,
            *[
                np.asarray(inputs[k], np.float32)
                for k in ("Wq", "Aq", "Bq", "Wk", "Ak", "Bk", "Wv", "Av", "Bv", "Wo", "Ao", "Bo")
            ],
        )
        return out, None

    in_maps = _prepare_in_maps(
        inputs["query"], inputs["key"], inputs["value"],
        inputs["Wq"], inputs["Aq"], inputs["Bq"],
        inputs["Wk"], inputs["Ak"], inputs["Bk"],
        inputs["Wv"], inputs["Av"], inputs["Bv"],
        inputs["Wo"], inputs["Ao"], inputs["Bo"],
    )
    nc = _get_nc()
    res = run_bass_kernel_spmd(
        nc, in_maps, core_ids=list(range(N_CORES)), trace=trace, **spmd_kwargs
    )
    out = np.empty((B, S, D), np.float32)
    for b in range(B):
        acc = res.results[CPG * b]["outT"].astype(np.float32)
        for g in range(1, CPG):
            acc = acc + res.results[CPG * b + g]["outT"]
        out[b] = acc.T
    return out, res


def kernel(**inputs):
    out, _ = run(inputs, trace=False)
    return out
